# revision 8
# baseline (speedup 1.0000x reference)
"""MoE-routed K-cluster autoencoder kernel for 8 Trainium2 NeuronCores.

Strategy
--------
Each row of x is reconstructed by the autoencoder of its kmeans cluster.
Computing all K experts densely for every row (like the reference) does
10x the needed matmul work, so we *route*:

  host:   sort rows by cluster, pack them into fixed-capacity "slots"
          (one cluster per slot; 4 slots x 1152 rows per core for uniform
          labels), pre-transpose so features lie on SBUF partitions, and
          flatten each row-chunk k-major so every DMA moves long
          contiguous runs.
  device: per slot, run the 6-layer MLP chain as feature-major matmuls
          (outT = W.T @ actsT). Weight-stationary groups stream all of a
          slot's row-chunks back-to-back; the next slot's encoder-0 work
          is software-pipelined into the current slot's serial mid-layers
          (the PE is in-order, so independent filler matmuls are emitted
          where the layer chain would stall); PSUM->SBUF bias+ReLU
          evictions alternate between ScalarE and VectorE so neither
          drain engine starves the PE (which would drop the HAM clock
          gate to 1.2 GHz); a short pre-warm matmul burst opens the clock
          gate while the first DMAs land.
  host:   scatter the per-slot outputs back to original row order.

Two operand-dtype modes (MODE): "bf16" (default -- half the x/weight DMA
bytes, fast weight loads, ~104us HW, ~5.6e-3 scale-relative absmax err)
and "f32r" (fp32 bytes end-to-end, ~177us, ~3.4e-4 err).
All shapes are static; the slot capacity config adapts to the label
histogram (uniform labels give the (4, 1152) config).
"""

import numpy as np

import concourse.tile as tile
from concourse import bacc, mybir
from concourse.bass_utils import run_bass_kernel_spmd

N_CORES = 8
B, D, H1, H2, L, K = 32768, 784, 256, 64, 16, 10
P = 112          # partition tile height for the D axis: 784 = 7 * 112
KT = D // P      # 7 k-tiles along D

# per-slot packed weight layout (column offsets in a [128, WSLOT] block)
_E0, _E1, _E2, _D0, _D1, _D2 = 0, 1792, 1920, 1936, 2000, 2256
WSLOT = 3824     # = 7*256 + 2*64 + 16 + 64 + 256 + 2*784
BSLOT = 14       # bias columns per slot: 2 + 1 + 1 + 1 + 2 + 7

# (slots_per_core, rows_per_slot)
_CONFIGS = [(4, 1152), (4, 1280), (8, 640), (16, 320), (32, 160)]

_F32 = mybir.dt.float32
_F32R = mybir.dt.float32r
_BF16 = mybir.dt.bfloat16
_RELU = mybir.ActivationFunctionType.Relu

# matmul-operand dtype mode: "bf16" (default, fast) or "f32r" (precise)
MODE = "bf16"


def _mdt_view(ap, mode):
    return ap.bitcast(_F32R) if mode == "f32r" else ap


def _chunks(R, mode="f32r"):
    """Split R rows into moving-operand chunks <=512.

    f32r: each chunk >=256 (the fp32r full-rate threshold), greedy 512s
    with a rebalanced tail.
    bf16: equal thirds/halves -- every matmul's moving dim stays large
    enough (>=290) that its weight load hides under the stream."""
    if mode == "bf16":
        n = max(1, (R + 511) // 512)
        base, extra = divmod(R, n)
        return [base + (1 if i < extra else 0) for i in range(n)]
    out, rem = [], R
    while rem > 0:
        c = min(512, rem)
        if c == 512 and 0 < rem - c < 256:
            c = max(256, min(512, (rem + 1) // 2))
        out.append(c)
        rem -= c
    return out


def _build_program(S, R, mode):
    mdt = _F32R if mode == "f32r" else _BF16
    idt = _F32 if mode == "f32r" else _BF16
    pipelined = mode == "bf16"
    ncols = S * R
    nflat = ncols * KT  # x/y are stored chunk-flattened: [P, sum(KT*nch)]
    nc = bacc.Bacc("TRN2", target_bir_lowering=False, debug=False)
    xt = nc.dram_tensor("xt", [P, nflat], idt, kind="ExternalInput").ap()
    wp = nc.dram_tensor("wp", [128, S * WSLOT], idt, kind="ExternalInput").ap()
    bp = nc.dram_tensor("bp", [128, S * BSLOT], _F32, kind="ExternalInput").ap()
    yt = nc.dram_tensor("yt", [P, nflat], idt, kind="ExternalOutput").ap()

    chunks = _chunks(R, mode)
    NCH = len(chunks)
    XS_BUFS = 3 if pipelined else 2
    W_BUFS = 4 if pipelined else 2
    H1_BUFS = 10 if pipelined else 6
    SM_BUFS = 6 if pipelined else 3

    with tile.TileContext(nc) as tc:
        with (
            tc.tile_pool(name="wpool", bufs=1) as wpool,
            tc.tile_pool(name="iopool", bufs=1) as iopool,
            tc.tile_pool(name="apool", bufs=1) as apool,
            tc.tile_pool(name="pspool", bufs=1, space="PSUM") as pspool,
        ):

            bsb = wpool.tile([128, S * BSLOT], _F32, tag="b", name="bsb",
                             bufs=1)
            wu = wpool.tile([128, 512], _BF16, tag="wu", name="wu", bufs=1)
            wups = [pspool.tile([128, 512], _F32, tag="ps", name="wups",
                                bufs=8) for _ in range(4)]

            def bias(lo, col):
                return bsb[0:lo, col:col + 1]

            def ps_tile(parts, nch):
                return pspool.tile([parts, nch], _F32, tag="ps", name="ps",
                                   bufs=8)

            drain_i = [0]

            def drain_relu(out, ps, bias_ap):
                """bias+ReLU PSUM->SBUF eviction, alternating ACT/DVE."""
                drain_i[0] += 1
                if drain_i[0] % 2:
                    nc.scalar.activation(out, ps, _RELU, bias=bias_ap)
                else:
                    nc.vector.tensor_scalar(out, ps, bias_ap, 0.0,
                                            mybir.AluOpType.add,
                                            mybir.AluOpType.max)

            def drain_bias(out, ps, bias_ap):
                """bias-only PSUM->SBUF eviction, alternating ACT/DVE."""
                drain_i[0] += 1
                if drain_i[0] % 2:
                    nc.scalar.add(out, ps, bias_ap)
                else:
                    nc.vector.tensor_scalar_add(out, ps, bias_ap)

            res = {}
            loff = []
            cum = 0
            for nch in chunks:
                loff.append(cum)
                cum += nch * KT

            def ensure_slot(s):
                """Allocate slot s's weight/x tiles and issue their DMAs.

                DMA wall cost is dominated by packet count (one packet per
                partition row per dma_start), so steady-state slots use one
                wide DMA per tensor.  Slot 0 is the startup critical path:
                its DMAs are split so the first e0 matmul's dependencies
                (e0 weights + chunk-0 x) land first."""
                if s in res or s >= S:
                    return
                w = wpool.tile([128, WSLOT], mdt, tag="w", name="w",
                               bufs=W_BUFS)
                xs = iopool.tile([128, R * KT], mdt, tag="xs", name="xs",
                                 bufs=XS_BUFS)
                base = s * R * KT
                if s == 0:
                    nc.sync.dma_start(
                        out=w[:, 0:_E1],
                        in_=_mdt_view(wp[:, 0:_E1], mode))
                    c1 = loff[1] if NCH > 1 else R * KT
                    nc.sync.dma_start(
                        out=xs[0:P, 0:c1],
                        in_=_mdt_view(xt[:, base:base + c1], mode))
                    nc.sync.dma_start(out=bsb, in_=bp)
                    if NCH > 1:
                        nc.sync.dma_start(
                            out=xs[0:P, c1:R * KT],
                            in_=_mdt_view(xt[:, base + c1:base + R * KT],
                                          mode))
                    nc.sync.dma_start(
                        out=w[:, _E1:],
                        in_=_mdt_view(wp[:, _E1:WSLOT], mode))
                else:
                    nc.sync.dma_start(
                        out=w,
                        in_=_mdt_view(wp[:, s * WSLOT:(s + 1) * WSLOT], mode))
                    nc.sync.dma_start(
                        out=xs[0:P],
                        in_=_mdt_view(xt[:, base:base + R * KT], mode))
                res[s] = {"w": w, "xs": xs, "base": base, "bb": s * BSLOT,
                          "h1": [[None] * NCH, [None] * NCH],
                          "e0ps": [None, None]}

            def xap(r, ci, k):
                nch = chunks[ci]
                c0 = loff[ci] + k * nch
                return r["xs"][0:P, c0:c0 + nch]

            def e0_group(s, m, k):
                """One weight-stationary e0 group: [784->256] m-half, k-tile."""
                r = res[s]
                if k == 0:
                    r["e0ps"][m] = [ps_tile(128, nch) for nch in chunks]
                wk = r["w"][0:P, _E0 + k * 256 + 128 * m:
                            _E0 + k * 256 + 128 * m + 128]
                for ci, nch in enumerate(chunks):
                    nc.tensor.matmul(r["e0ps"][m][ci], wk, xap(r, ci, k),
                                     start=(k == 0), stop=(k == KT - 1))
                if k == KT - 1:
                    for ci, nch in enumerate(chunks):
                        t = apool.tile([128, nch], mdt, tag="h1", name="h1",
                                       bufs=H1_BUFS)
                        drain_relu(t, r["e0ps"][m][ci], bias(128, r["bb"] + m))
                        r["h1"][m][ci] = t
                    r["e0ps"][m] = None

            E0_ORDER = [(m, k) for m in range(2) for k in range(KT)]

            if pipelined:
                ensure_slot(0)
                # PE pre-warm: throwaway matmuls on a zeroed tile open the
                # HAM clock-gate to 2.4 GHz and occupy the PE for ~the time
                # the first e0 weight/x DMAs need to land (~6us).
                nc.vector.memset(wu, 0)
                for i in range(12):
                    nc.tensor.matmul(wups[i % 4], wu[:, 0:128], wu,
                                     start=True, stop=True)
                for i in range(6):
                    nc.tensor.matmul(wups[i % 4][:, 0:128], wu[:, 0:128],
                                     wu[:, 0:128], start=True, stop=True)
                ensure_slot(1)
                # slot 0 runs chunk-major: each chunk's e0 needs only that
                # chunk's x slice, so compute self-paces with the head DMAs
                r0 = res[0]
                for ci, nch in enumerate(chunks):
                    for m in range(2):
                        ps0 = ps_tile(128, nch)
                        for k in range(KT):
                            wk = r0["w"][0:P, _E0 + k * 256 + 128 * m:
                                         _E0 + k * 256 + 128 * m + 128]
                            nc.tensor.matmul(ps0, wk, xap(r0, ci, k),
                                             start=(k == 0), stop=(k == KT - 1))
                        t = apool.tile([128, nch], mdt, tag="h1", name="h1",
                                       bufs=H1_BUFS)
                        drain_relu(t, ps0, bias(128, r0["bb"] + m))
                        r0["h1"][m][ci] = t

            for s in range(S):
                if pipelined:
                    ensure_slot(s + 2)
                    filler = iter(E0_ORDER) if s + 1 < S else iter([])
                else:
                    ensure_slot(s)
                    for m, k in E0_ORDER:
                        e0_group(s, m, k)
                    filler = iter([])

                def fill(n):
                    for _ in range(n):
                        mk = next(filler, None)
                        if mk is not None:
                            e0_group(s + 1, *mk)

                r = res[s]
                w, bb, base, h1 = r["w"], r["bb"], r["base"], r["h1"]

                # encoder 1: [256 -> 64]
                ps = [None] * NCH
                for k in range(2):
                    wk = w[0:128, _E1 + 64 * k:_E1 + 64 * k + 64]
                    for ci, nch in enumerate(chunks):
                        if k == 0:
                            ps[ci] = ps_tile(64, nch)
                        nc.tensor.matmul(ps[ci], wk, h1[k][ci],
                                         start=(k == 0), stop=(k == 1))
                h2 = []
                for ci, nch in enumerate(chunks):
                    t = apool.tile([64, nch], mdt, tag="h2", name="h2", bufs=SM_BUFS)
                    drain_relu(t, ps[ci], bias(64, bb + 2))
                    h2.append(t)
                fill(2)

                # encoder 2: [64 -> 16]
                ps = [None] * NCH
                wk = w[0:64, _E2:_E2 + 16]
                for ci, nch in enumerate(chunks):
                    ps[ci] = ps_tile(16, nch)
                    nc.tensor.matmul(ps[ci], wk, h2[ci], start=True, stop=True)
                z = []
                for ci, nch in enumerate(chunks):
                    t = apool.tile([16, nch], mdt, tag="z", name="z", bufs=SM_BUFS)
                    drain_relu(t, ps[ci], bias(16, bb + 3))
                    z.append(t)
                fill(2)

                # decoder 0: [16 -> 64]
                ps = [None] * NCH
                wk = w[0:16, _D0:_D0 + 64]
                for ci, nch in enumerate(chunks):
                    ps[ci] = ps_tile(64, nch)
                    nc.tensor.matmul(ps[ci], wk, z[ci], start=True, stop=True)
                a1 = []
                for ci, nch in enumerate(chunks):
                    t = apool.tile([64, nch], mdt, tag="a1", name="a1", bufs=SM_BUFS)
                    drain_relu(t, ps[ci], bias(64, bb + 4))
                    a1.append(t)
                fill(2)

                # decoder 1: [64 -> 256]
                a2 = [[None] * NCH, [None] * NCH]
                for m in range(2):
                    wk = w[0:64, _D1 + 128 * m:_D1 + 128 * m + 128]
                    ps = [None] * NCH
                    for ci, nch in enumerate(chunks):
                        ps[ci] = ps_tile(128, nch)
                        nc.tensor.matmul(ps[ci], wk, a1[ci],
                                         start=True, stop=True)
                    for ci, nch in enumerate(chunks):
                        t = apool.tile([128, nch], mdt, tag="a2", name="a2",
                                       bufs=7)
                        drain_relu(t, ps[ci], bias(128, bb + 5 + m))
                        a2[m][ci] = t
                    fill(2)

                # decoder 2: [256 -> 784], bias only.  The last slot runs
                # in two chunk-groups so the bulk of its y writeback DMA
                # overlaps the final chunk's matmuls instead of all of it
                # serializing after the last drain.
                ys = iopool.tile([128, R * KT], idt, tag="ys", name="ys",
                                 bufs=2)
                if s == S - 1 and NCH > 1:
                    groups = [list(range(NCH - 1)), [NCH - 1]]
                else:
                    groups = [list(range(NCH))]
                for gi, grp in enumerate(groups):
                    for mm in range(KT):
                        ps = [None] * NCH
                        for k in range(2):
                            wk = w[0:128, _D2 + 784 * k + 112 * mm:
                                   _D2 + 784 * k + 112 * mm + 112]
                            for ci in grp:
                                nch = chunks[ci]
                                if k == 0:
                                    ps[ci] = ps_tile(112, nch)
                                nc.tensor.matmul(ps[ci], wk, a2[k][ci],
                                                 start=(k == 0), stop=(k == 1))
                        for ci in grp:
                            nch = chunks[ci]
                            c0 = loff[ci] + mm * nch
                            drain_bias(ys[0:P, c0:c0 + nch], ps[ci],
                                       bias(112, bb + 7 + mm))
                        if gi == 0 and mm < 4:
                            fill(1)
                    lo = loff[grp[0]]
                    hi = loff[grp[-1]] + KT * chunks[grp[-1]]
                    nc.sync.dma_start(out=yt[:, base + lo:base + hi],
                                      in_=ys[0:P, lo:hi])
                fill(14)
                del res[s]
    nc.compile()
    return nc


_programs = {}


def _get_program(S, R, mode):
    if (S, R, mode) not in _programs:
        _programs[(S, R, mode)] = _build_program(S, R, mode)
    return _programs[(S, R, mode)]


def _pack_weights(params, slot_clusters):
    S = len(slot_clusters)
    wpk = np.zeros((128, S * WSLOT), np.float32)
    bpk = np.zeros((128, S * BSLOT), np.float32)
    for s, c in enumerate(slot_clusters):
        wb, bb = s * WSLOT, s * BSLOT
        we0, we1, we2 = params["w_e0"][c], params["w_e1"][c], params["w_e2"][c]
        wd0, wd1, wd2 = params["w_d0"][c], params["w_d1"][c], params["w_d2"][c]
        for k in range(KT):
            wpk[0:P, wb + _E0 + k * 256: wb + _E0 + (k + 1) * 256] = \
                we0[P * k:P * (k + 1), :]
        for k in range(2):
            wpk[0:128, wb + _E1 + 64 * k: wb + _E1 + 64 * (k + 1)] = \
                we1[128 * k:128 * (k + 1), :]
        wpk[0:64, wb + _E2: wb + _E2 + 16] = we2
        wpk[0:16, wb + _D0: wb + _D0 + 64] = wd0
        wpk[0:64, wb + _D1: wb + _D1 + 256] = wd1
        for k in range(2):
            wpk[0:128, wb + _D2 + 784 * k: wb + _D2 + 784 * (k + 1)] = \
                wd2[128 * k:128 * (k + 1), :]

        be0, be1, be2 = params["b_e0"][c], params["b_e1"][c], params["b_e2"][c]
        bd0, bd1, bd2 = params["b_d0"][c], params["b_d1"][c], params["b_d2"][c]
        bpk[0:128, bb + 0] = be0[0:128]
        bpk[0:128, bb + 1] = be0[128:256]
        bpk[0:64, bb + 2] = be1
        bpk[0:16, bb + 3] = be2
        bpk[0:64, bb + 4] = bd0
        bpk[0:128, bb + 5] = bd1[0:128]
        bpk[0:128, bb + 6] = bd1[128:256]
        for m in range(KT):
            bpk[0:P, bb + 7 + m] = bd2[P * m:P * (m + 1)]
    return wpk, bpk


def _route(labels, mode):
    """Assign rows to (core, slot) blocks; returns config + per-slot rows."""
    counts = np.bincount(labels, minlength=K)
    configs = _CONFIGS if mode == "bf16" else _CONFIGS[1:]
    for S, R in configs:
        need = int(np.sum((counts + R - 1) // R))
        if need <= N_CORES * S:
            break
    nslots = N_CORES * S
    order = np.argsort(labels, kind="stable")
    slot_cluster = np.zeros(nslots, np.int64)
    slot_rows = [np.empty(0, np.int64)] * nslots
    si = pos = 0
    for c in range(K):
        cnt = int(counts[c])
        rows_c = order[pos:pos + cnt]
        pos += cnt
        for off in range(0, cnt, R):
            slot_cluster[si] = c
            slot_rows[si] = rows_c[off:off + R]
            si += 1
    return S, R, slot_cluster, slot_rows


def _flatten_xcore(xcore_t, R, chunks):
    """[D, S*R] feature-major slab -> chunk-flattened [P, S*R*KT]."""
    ncols = xcore_t.shape[1]
    S = ncols // R
    flat = np.empty((P, ncols * KT), np.float32)
    pos = 0
    for s in range(S):
        col = s * R
        for nch in chunks:
            blk = xcore_t[:, col:col + nch]              # [784, nch]
            blk = blk.reshape(KT, P, nch).transpose(1, 0, 2)  # [P, KT, nch]
            flat[:, pos:pos + KT * nch] = blk.reshape(P, KT * nch)
            pos += KT * nch
            col += nch
    return flat


def _unflatten_ycore(yflat, R, chunks):
    """chunk-flattened [P, S*R*KT] -> row-major [S*R, D]."""
    ncols = yflat.shape[1] // KT
    S = ncols // R
    out = np.empty((ncols, D), np.float32)
    pos = 0
    for s in range(S):
        col = s * R
        for nch in chunks:
            blk = yflat[:, pos:pos + KT * nch].reshape(P, KT, nch)
            out[col:col + nch] = blk.transpose(2, 1, 0).reshape(nch, D)
            pos += KT * nch
            col += nch
    return out


def kernel_traced(inputs, trace=False, mode=None):
    if mode is None:
        mode = MODE
    x = np.ascontiguousarray(np.asarray(inputs["x"], dtype=np.float32))
    labels = np.asarray(inputs["kmeans_label"]).astype(np.int64).ravel()
    params = {k: np.asarray(v, dtype=np.float32)
              for k, v in inputs.items() if k not in ("x", "kmeans_label")}

    S, R, slot_cluster, slot_rows = _route(labels, mode)
    chunks = _chunks(R, mode)
    nc = _get_program(S, R, mode)

    in_maps = []
    for i in range(N_CORES):
        xcore = np.zeros((S * R, D), np.float32)
        for s in range(S):
            rows = slot_rows[i * S + s]
            if len(rows):
                xcore[s * R: s * R + len(rows)] = x[rows]
        wpk, bpk = _pack_weights(params, slot_cluster[i * S:(i + 1) * S])
        xflat = _flatten_xcore(np.ascontiguousarray(xcore.T), R, chunks)
        if mode == "bf16":
            import ml_dtypes
            xflat = xflat.astype(ml_dtypes.bfloat16)
            wpk = wpk.astype(ml_dtypes.bfloat16)
        in_maps.append({"xt": xflat, "wp": wpk, "bp": bpk})

    res = run_bass_kernel_spmd(nc, in_maps, core_ids=list(range(N_CORES)),
                               trace=trace)

    out = np.zeros_like(x)
    for i in range(N_CORES):
        yraw = np.asarray(res.results[i]["yt"]).astype(np.float32)
        ytT = _unflatten_ycore(yraw, R, chunks)
        for s in range(S):
            rows = slot_rows[i * S + s]
            if len(rows):
                out[rows] = ytT[s * R: s * R + len(rows)]
    return out, res


def kernel(**inputs):
    out, _ = kernel_traced(inputs, trace=False)
    return out



# revision 10
# speedup vs baseline: 1.0819x; 1.0819x over previous
"""MoE-routed K-cluster autoencoder kernel for 8 Trainium2 NeuronCores.

Strategy
--------
Each row of x is reconstructed by the autoencoder of its kmeans cluster.
Computing all K experts densely for every row (like the reference) does
10x the needed matmul work, so we *route*:

  host:   sort rows by cluster, pack them into fixed-capacity "slots"
          (one cluster per slot; 4 slots x 1152 rows per core for uniform
          labels), pre-transpose so features lie on SBUF partitions, and
          flatten each row-chunk k-major so every DMA moves long
          contiguous runs.
  device: per slot, run the 6-layer MLP chain as feature-major matmuls
          (outT = W.T @ actsT). Weight-stationary groups stream all of a
          slot's row-chunks back-to-back; the next slot's encoder-0 work
          is software-pipelined into the current slot's serial mid-layers
          (the PE is in-order, so independent filler matmuls are emitted
          where the layer chain would stall, and cheap "dummy" matmuls
          are emitted when no filler is left so the HAM clock gate never
          drops the PE to 1.2 GHz); PSUM->SBUF bias+ReLU evictions
          alternate between ScalarE and VectorE.
  host:   scatter the per-slot outputs back to original row order.

The kernel is simultaneously PE-bound (~65us of matmul streaming) and
DMA-bound (~65us at the ~270 GB/s effective per-core rate), so the
single in-order DMA queue is sequenced just-in-time: slot 0/1 tensors
are split so each consumer's bytes arrive right before its matmuls,
later slots stream as whole-slot transfers, and the last slot's y
writeback is chunk-major so only the final ~0.6 MB trails the last
matmul.

Two operand-dtype modes (MODE): "bf16" (default -- half the x/weight DMA
bytes, ~5.6e-3 scale-relative absmax err) and "f32r" (fp32 bytes
end-to-end, slower, ~3.4e-4 err).
"""

import numpy as np

import concourse.tile as tile
from concourse import bacc, mybir
from concourse.bass_utils import run_bass_kernel_spmd

N_CORES = 8
B, D, H1, H2, L, K = 32768, 784, 256, 64, 16, 10
P = 112          # partition tile height for the D axis: 784 = 7 * 112
KT = D // P      # 7 k-tiles along D

# per-slot packed weight layout (column offsets in a [128, WSLOT] block)
_E0, _E1, _E2, _D0, _D1, _D2 = 0, 1792, 1920, 1936, 2000, 2256
WSLOT = 3824     # = 7*256 + 2*64 + 16 + 64 + 256 + 2*784
BSLOT = 14       # bias columns per slot: 2 + 1 + 1 + 1 + 2 + 7

# (slots_per_core, rows_per_slot)
_CONFIGS = [(4, 1152), (4, 1280), (8, 640), (16, 320), (32, 160)]

_F32 = mybir.dt.float32
_F32R = mybir.dt.float32r
_BF16 = mybir.dt.bfloat16
_RELU = mybir.ActivationFunctionType.Relu

# matmul-operand dtype mode: "bf16" (default, fast) or "f32r" (precise)
MODE = "bf16"


def _mdt_view(ap, mode):
    return ap.bitcast(_F32R) if mode == "f32r" else ap


def _chunks(R, mode="f32r"):
    """Split R rows into moving-operand chunks <=512.

    f32r: each chunk >=256 (the fp32r full-rate threshold), greedy 512s
    with a rebalanced tail.
    bf16: equal thirds/halves -- every matmul's moving dim stays large
    enough (>=290) that its weight load hides under the stream."""
    if mode == "bf16":
        n = max(1, (R + 511) // 512)
        base, extra = divmod(R, n)
        return [base + (1 if i < extra else 0) for i in range(n)]
    out, rem = [], R
    while rem > 0:
        c = min(512, rem)
        if c == 512 and 0 < rem - c < 256:
            c = max(256, min(512, (rem + 1) // 2))
        out.append(c)
        rem -= c
    return out


def _build_program(S, R, mode):
    mdt = _F32R if mode == "f32r" else _BF16
    idt = _F32 if mode == "f32r" else _BF16
    pipelined = mode == "bf16"
    ncols = S * R
    nflat = ncols * KT  # x/y are stored chunk-flattened: [P, sum(KT*nch)]
    nc = bacc.Bacc("TRN2", target_bir_lowering=False, debug=False)
    xt = nc.dram_tensor("xt", [P, nflat], idt, kind="ExternalInput").ap()
    wp = nc.dram_tensor("wp", [128, S * WSLOT], idt, kind="ExternalInput").ap()
    bp = nc.dram_tensor("bp", [128, S * BSLOT], _F32, kind="ExternalInput").ap()
    yt = nc.dram_tensor("yt", [P, nflat], idt, kind="ExternalOutput").ap()

    chunks = _chunks(R, mode)
    NCH = len(chunks)
    XS_BUFS = 3 if pipelined else 2
    W_BUFS = 4 if pipelined else 2
    H1_BUFS = 10 if pipelined else 6
    SM_BUFS = 6 if pipelined else 3

    with tile.TileContext(nc) as tc:
        with (
            tc.tile_pool(name="wpool", bufs=1) as wpool,
            tc.tile_pool(name="iopool", bufs=1) as iopool,
            tc.tile_pool(name="apool", bufs=1) as apool,
            tc.tile_pool(name="pspool", bufs=1, space="PSUM") as pspool,
        ):
            bsb = wpool.tile([128, S * BSLOT], _F32, tag="b", name="bsb",
                             bufs=1)
            wu = wpool.tile([128, 512], _BF16, tag="wu", name="wu", bufs=1)
            wups = [pspool.tile([128, 512], _F32, tag="ps", name="wups",
                                bufs=7) for _ in range(4)]
            # dedicated keep-warm PSUM target, outside the "ps" rotation:
            # dummy matmuls only ever WAW-chain on the in-order PE.
            dum = pspool.tile([128, 128], _F32, tag="dum", name="dum",
                              bufs=1)

            def dummy(n):
                """n cheap 128-col matmuls: keeps the PE HAM-warm where
                the schedule would otherwise leave the engine idle."""
                for _ in range(n):
                    nc.tensor.matmul(dum, wu[:, 0:128], wu[:, 0:128],
                                     start=True, stop=True)

            def bias(lo, col):
                return bsb[0:lo, col:col + 1]

            def ps_tile(parts, nch):
                return pspool.tile([parts, nch], _F32, tag="ps", name="ps",
                                   bufs=7)

            drain_i = [0]

            def drain_relu(out, ps, bias_ap):
                """bias+ReLU PSUM->SBUF eviction, alternating ACT/DVE."""
                drain_i[0] += 1
                if drain_i[0] % 2:
                    nc.scalar.activation(out, ps, _RELU, bias=bias_ap)
                else:
                    nc.vector.tensor_scalar(out, ps, bias_ap, 0.0,
                                            mybir.AluOpType.add,
                                            mybir.AluOpType.max)

            def drain_bias(out, ps, bias_ap):
                """bias-only PSUM->SBUF eviction, alternating ACT/DVE."""
                drain_i[0] += 1
                if drain_i[0] % 2:
                    nc.scalar.add(out, ps, bias_ap)
                else:
                    nc.vector.tensor_scalar_add(out, ps, bias_ap)

            res = {}
            loff = []
            cum = 0
            for nch in chunks:
                loff.append(cum)
                cum += nch * KT

            def dma_w(s, c0, c1):
                r = res[s]
                nc.sync.dma_start(
                    out=r["w"][:, c0:c1],
                    in_=_mdt_view(wp[:, s * WSLOT + c0:s * WSLOT + c1], mode))

            def dma_x(s, e0, e1):
                r = res[s]
                nc.sync.dma_start(
                    out=r["xs"][0:P, e0:e1],
                    in_=_mdt_view(xt[:, s * R * KT + e0:s * R * KT + e1],
                                  mode))

            def alloc_slot(s):
                if s in res or s >= S:
                    return None
                w = wpool.tile([128, WSLOT], mdt, tag="w", name="w",
                               bufs=W_BUFS)
                xs = iopool.tile([128, R * KT], mdt, tag="xs", name="xs",
                                 bufs=XS_BUFS)
                res[s] = {"w": w, "xs": xs, "base": s * R * KT,
                          "bb": s * BSLOT,
                          "h1": [[None] * NCH, [None] * NCH]}
                return res[s]

            def ensure_slot(s):
                """Steady-state slot prefetch: one wide DMA per tensor."""
                if s in res or s >= S:
                    return
                alloc_slot(s)
                dma_w(s, 0, WSLOT)
                dma_x(s, 0, R * KT)

            def xap(r, ci, k):
                nch = chunks[ci]
                c0 = loff[ci] + k * nch
                return r["xs"][0:P, c0:c0 + nch]

            def e0_unit(s, m, ci):
                """One filler unit: chunk ci's full e0 contraction for
                m-half m (7 matmuls, one short-lived PSUM bank)."""
                r = res[s]
                nch = chunks[ci]
                ps = ps_tile(128, nch)
                for k in range(KT):
                    wk = r["w"][0:P, _E0 + k * 256 + 128 * m:
                                _E0 + k * 256 + 128 * m + 128]
                    nc.tensor.matmul(ps, wk, xap(r, ci, k),
                                     start=(k == 0), stop=(k == KT - 1))
                t = apool.tile([128, nch], mdt, tag="h1", name="h1",
                               bufs=H1_BUFS)
                drain_relu(t, ps, bias(128, r["bb"] + m))
                r["h1"][m][ci] = t

            E0_ORDER = [(m, ci) for m in range(2) for ci in range(NCH)]

            if pipelined:
                # Startup: the DMA queue is in-order and ~270 GB/s, so
                # sequence slot-0/1 pieces just-in-time for the PE.
                alloc_slot(0)
                alloc_slot(1)
                dma_w(0, 0, _E1)                       # s0 e0 weights
                dma_x(0, 0, loff[1] if NCH > 1 else R * KT)   # s0 chunk 0
                nc.sync.dma_start(out=bsb, in_=bp)     # biases (drains)
                dma_w(1, 0, _E1)                       # s1 e0 weights
                for ci in range(1, NCH):               # s0 chunks 1..
                    dma_x(0, loff[ci], loff[ci] + KT * chunks[ci])
                dma_w(0, _E1, _D2)                     # s0 mid weights
                if NCH > 1:                            # s1 chunk 0
                    dma_x(1, 0, loff[1])
                dma_w(0, _D2, WSLOT)                   # s0 d2 weights
                if NCH > 1:                            # s1 chunks 1..
                    dma_x(1, loff[1], R * KT)
                else:
                    dma_x(1, 0, R * KT)
                dma_w(1, _E1, WSLOT)                   # s1 rest

                # PE pre-warm: open the HAM clock gate while DMAs land.
                nc.vector.memset(wu, 0)
                for i in range(12):
                    nc.tensor.matmul(wups[i % 4], wu[:, 0:128], wu,
                                     start=True, stop=True)
                dummy(6)

                # slot 0 runs chunk-major, self-pacing with the head DMAs
                r0 = res[0]
                for ci in range(NCH):
                    for m in range(2):
                        e0_unit(0, m, ci)
                    if ci + 1 < NCH:
                        dummy(6)
            for s in range(S):
                if pipelined:
                    ensure_slot(s + 2)
                    filler = iter(E0_ORDER) if s + 1 < S else iter([])
                else:
                    if s == 0:
                        nc.sync.dma_start(out=bsb, in_=bp)
                    ensure_slot(s)
                    for m, ci in E0_ORDER:
                        e0_unit(s, m, ci)
                    filler = iter([])

                def fill(n, pad=True):
                    for _ in range(n):
                        mk = next(filler, None)
                        if mk is not None:
                            e0_unit(s + 1, *mk)
                        elif pad and pipelined:
                            dummy(3)

                r = res[s]
                w, bb, base, h1 = r["w"], r["bb"], r["base"], r["h1"]

                # encoder 1: [256 -> 64]
                ps = [None] * NCH
                for k in range(2):
                    wk = w[0:128, _E1 + 64 * k:_E1 + 64 * k + 64]
                    for ci, nch in enumerate(chunks):
                        if k == 0:
                            ps[ci] = ps_tile(64, nch)
                        nc.tensor.matmul(ps[ci], wk, h1[k][ci],
                                         start=(k == 0), stop=(k == 1))
                h2 = []
                for ci, nch in enumerate(chunks):
                    t = apool.tile([64, nch], mdt, tag="h2", name="h2", bufs=SM_BUFS)
                    drain_relu(t, ps[ci], bias(64, bb + 2))
                    h2.append(t)
                fill(1)

                # encoder 2: [64 -> 16]
                ps = [None] * NCH
                wk = w[0:64, _E2:_E2 + 16]
                for ci, nch in enumerate(chunks):
                    ps[ci] = ps_tile(16, nch)
                    nc.tensor.matmul(ps[ci], wk, h2[ci], start=True, stop=True)
                z = []
                for ci, nch in enumerate(chunks):
                    t = apool.tile([16, nch], mdt, tag="z", name="z", bufs=SM_BUFS)
                    drain_relu(t, ps[ci], bias(16, bb + 3))
                    z.append(t)
                fill(1)

                # decoder 0: [16 -> 64]
                ps = [None] * NCH
                wk = w[0:16, _D0:_D0 + 64]
                for ci, nch in enumerate(chunks):
                    ps[ci] = ps_tile(64, nch)
                    nc.tensor.matmul(ps[ci], wk, z[ci], start=True, stop=True)
                a1 = []
                for ci, nch in enumerate(chunks):
                    t = apool.tile([64, nch], mdt, tag="a1", name="a1", bufs=SM_BUFS)
                    drain_relu(t, ps[ci], bias(64, bb + 4))
                    a1.append(t)
                fill(1)

                # decoder 1: [64 -> 256]
                a2 = [[None] * NCH, [None] * NCH]
                for m in range(2):
                    wk = w[0:64, _D1 + 128 * m:_D1 + 128 * m + 128]
                    ps = [None] * NCH
                    for ci, nch in enumerate(chunks):
                        ps[ci] = ps_tile(128, nch)
                        nc.tensor.matmul(ps[ci], wk, a1[ci],
                                         start=True, stop=True)
                    for ci, nch in enumerate(chunks):
                        t = apool.tile([128, nch], mdt, tag="a2", name="a2",
                                       bufs=7)
                        drain_relu(t, ps[ci], bias(128, bb + 5 + m))
                        a2[m][ci] = t
                    fill(1)

                # decoder 2: [256 -> 784], bias only.
                ys = iopool.tile([128, R * KT], idt, tag="ys", name="ys",
                                 bufs=2)
                if s == S - 1:
                    # last slot: chunk-major so each chunk's y writeback
                    # streams while the next chunk's matmuls run; only the
                    # final chunk's y trails the last matmul.
                    for ci, nch in enumerate(chunks):
                        for mm in range(KT):
                            ps = ps_tile(112, nch)
                            for k in range(2):
                                wk = w[0:128, _D2 + 784 * k + 112 * mm:
                                       _D2 + 784 * k + 112 * mm + 112]
                                nc.tensor.matmul(ps, wk, a2[k][ci],
                                                 start=(k == 0), stop=(k == 1))
                            c0 = loff[ci] + mm * nch
                            drain_bias(ys[0:P, c0:c0 + nch], ps,
                                       bias(112, bb + 7 + mm))
                        lo = loff[ci]
                        nc.sync.dma_start(
                            out=yt[:, base + lo:base + lo + KT * nch],
                            in_=ys[0:P, lo:lo + KT * nch])
                else:
                    for mm in range(KT):
                        ps = [None] * NCH
                        for k in range(2):
                            wk = w[0:128, _D2 + 784 * k + 112 * mm:
                                   _D2 + 784 * k + 112 * mm + 112]
                            for ci, nch in enumerate(chunks):
                                if k == 0:
                                    ps[ci] = ps_tile(112, nch)
                                nc.tensor.matmul(ps[ci], wk, a2[k][ci],
                                                 start=(k == 0), stop=(k == 1))
                        for ci, nch in enumerate(chunks):
                            c0 = loff[ci] + mm * nch
                            drain_bias(ys[0:P, c0:c0 + nch], ps[ci],
                                       bias(112, bb + 7 + mm))
                        if mm in (0, 2):
                            fill(1)
                    nc.sync.dma_start(out=yt[:, base:base + R * KT],
                                      in_=ys[0:P])
                fill(6, pad=False)
                del res[s]
    nc.compile()
    return nc


_programs = {}


def _get_program(S, R, mode):
    if (S, R, mode) not in _programs:
        _programs[(S, R, mode)] = _build_program(S, R, mode)
    return _programs[(S, R, mode)]


def _pack_weights(params, slot_clusters):
    S = len(slot_clusters)
    wpk = np.zeros((128, S * WSLOT), np.float32)
    bpk = np.zeros((128, S * BSLOT), np.float32)
    for s, c in enumerate(slot_clusters):
        wb, bb = s * WSLOT, s * BSLOT
        we0, we1, we2 = params["w_e0"][c], params["w_e1"][c], params["w_e2"][c]
        wd0, wd1, wd2 = params["w_d0"][c], params["w_d1"][c], params["w_d2"][c]
        for k in range(KT):
            wpk[0:P, wb + _E0 + k * 256: wb + _E0 + (k + 1) * 256] = \
                we0[P * k:P * (k + 1), :]
        for k in range(2):
            wpk[0:128, wb + _E1 + 64 * k: wb + _E1 + 64 * (k + 1)] = \
                we1[128 * k:128 * (k + 1), :]
        wpk[0:64, wb + _E2: wb + _E2 + 16] = we2
        wpk[0:16, wb + _D0: wb + _D0 + 64] = wd0
        wpk[0:64, wb + _D1: wb + _D1 + 256] = wd1
        for k in range(2):
            wpk[0:128, wb + _D2 + 784 * k: wb + _D2 + 784 * (k + 1)] = \
                wd2[128 * k:128 * (k + 1), :]

        be0, be1, be2 = params["b_e0"][c], params["b_e1"][c], params["b_e2"][c]
        bd0, bd1, bd2 = params["b_d0"][c], params["b_d1"][c], params["b_d2"][c]
        bpk[0:128, bb + 0] = be0[0:128]
        bpk[0:128, bb + 1] = be0[128:256]
        bpk[0:64, bb + 2] = be1
        bpk[0:16, bb + 3] = be2
        bpk[0:64, bb + 4] = bd0
        bpk[0:128, bb + 5] = bd1[0:128]
        bpk[0:128, bb + 6] = bd1[128:256]
        for m in range(KT):
            bpk[0:P, bb + 7 + m] = bd2[P * m:P * (m + 1)]
    return wpk, bpk


def _route(labels, mode):
    """Assign rows to (core, slot) blocks; returns config + per-slot rows."""
    counts = np.bincount(labels, minlength=K)
    configs = _CONFIGS if mode == "bf16" else _CONFIGS[1:]
    for S, R in configs:
        need = int(np.sum((counts + R - 1) // R))
        if need <= N_CORES * S:
            break
    nslots = N_CORES * S
    order = np.argsort(labels, kind="stable")
    slot_cluster = np.zeros(nslots, np.int64)
    slot_rows = [np.empty(0, np.int64)] * nslots
    si = pos = 0
    for c in range(K):
        cnt = int(counts[c])
        rows_c = order[pos:pos + cnt]
        pos += cnt
        for off in range(0, cnt, R):
            slot_cluster[si] = c
            slot_rows[si] = rows_c[off:off + R]
            si += 1
    return S, R, slot_cluster, slot_rows


def _flatten_xcore(xcore_t, R, chunks):
    """[D, S*R] feature-major slab -> chunk-flattened [P, S*R*KT]."""
    ncols = xcore_t.shape[1]
    S = ncols // R
    flat = np.empty((P, ncols * KT), np.float32)
    pos = 0
    for s in range(S):
        col = s * R
        for nch in chunks:
            blk = xcore_t[:, col:col + nch]              # [784, nch]
            blk = blk.reshape(KT, P, nch).transpose(1, 0, 2)  # [P, KT, nch]
            flat[:, pos:pos + KT * nch] = blk.reshape(P, KT * nch)
            pos += KT * nch
            col += nch
    return flat


def _unflatten_ycore(yflat, R, chunks):
    """chunk-flattened [P, S*R*KT] -> row-major [S*R, D]."""
    ncols = yflat.shape[1] // KT
    S = ncols // R
    out = np.empty((ncols, D), np.float32)
    pos = 0
    for s in range(S):
        col = s * R
        for nch in chunks:
            blk = yflat[:, pos:pos + KT * nch].reshape(P, KT, nch)
            out[col:col + nch] = blk.transpose(2, 1, 0).reshape(nch, D)
            pos += KT * nch
            col += nch
    return out


def kernel_traced(inputs, trace=False, mode=None):
    if mode is None:
        mode = MODE
    x = np.ascontiguousarray(np.asarray(inputs["x"], dtype=np.float32))
    labels = np.asarray(inputs["kmeans_label"]).astype(np.int64).ravel()
    params = {k: np.asarray(v, dtype=np.float32)
              for k, v in inputs.items() if k not in ("x", "kmeans_label")}

    S, R, slot_cluster, slot_rows = _route(labels, mode)
    chunks = _chunks(R, mode)
    nc = _get_program(S, R, mode)

    in_maps = []
    for i in range(N_CORES):
        xcore = np.zeros((S * R, D), np.float32)
        for s in range(S):
            rows = slot_rows[i * S + s]
            if len(rows):
                xcore[s * R: s * R + len(rows)] = x[rows]
        wpk, bpk = _pack_weights(params, slot_cluster[i * S:(i + 1) * S])
        xflat = _flatten_xcore(np.ascontiguousarray(xcore.T), R, chunks)
        if mode == "bf16":
            import ml_dtypes
            xflat = xflat.astype(ml_dtypes.bfloat16)
            wpk = wpk.astype(ml_dtypes.bfloat16)
        in_maps.append({"xt": xflat, "wp": wpk, "bp": bpk})

    res = run_bass_kernel_spmd(nc, in_maps, core_ids=list(range(N_CORES)),
                               trace=trace)

    out = np.zeros_like(x)
    for i in range(N_CORES):
        yraw = np.asarray(res.results[i]["yt"]).astype(np.float32)
        ytT = _unflatten_ycore(yraw, R, chunks)
        for s in range(S):
            rows = slot_rows[i * S + s]
            if len(rows):
                out[rows] = ytT[s * R: s * R + len(rows)]
    return out, res


def kernel(**inputs):
    out, _ = kernel_traced(inputs, trace=False)
    return out


# revision 11
# speedup vs baseline: 1.0922x; 1.0095x over previous
"""MoE-routed K-cluster autoencoder kernel for 8 Trainium2 NeuronCores.

Strategy
--------
Each row of x is reconstructed by the autoencoder of its kmeans cluster.
Computing all K experts densely for every row (like the reference) does
10x the needed matmul work, so we *route*:

  host:   sort rows by cluster, pack them into fixed-capacity "slots"
          (one cluster per slot; 4 slots x 1152 rows per core for uniform
          labels), pre-transpose so features lie on SBUF partitions, and
          flatten each row-chunk k-major so every DMA moves long
          contiguous runs.
  device: per slot, run the 6-layer MLP chain as feature-major matmuls
          (outT = W.T @ actsT). Weight-stationary groups stream all of a
          slot's row-chunks back-to-back; the next slot's encoder-0 work
          is software-pipelined into the current slot's serial mid-layers
          (the PE is in-order, so independent filler matmuls are emitted
          where the layer chain would stall, and cheap "dummy" matmuls
          are emitted when no filler is left so the HAM clock gate never
          drops the PE to 1.2 GHz); PSUM->SBUF bias+ReLU evictions
          alternate between ScalarE and VectorE.
  host:   scatter the per-slot outputs back to original row order.

The kernel is simultaneously PE-bound (~65us of matmul streaming) and
DMA-bound (~65us at the ~270 GB/s effective per-core rate), so the
single in-order DMA queue is sequenced just-in-time: slot 0/1 tensors
are split so each consumer's bytes arrive right before its matmuls,
later slots stream as whole-slot transfers, and the last slot's y
writeback is chunk-major so only the final ~0.6 MB trails the last
matmul.

Two operand-dtype modes (MODE): "bf16" (default -- half the x/weight DMA
bytes, ~5.6e-3 scale-relative absmax err) and "f32r" (fp32 bytes
end-to-end, slower, ~3.4e-4 err).
"""

import numpy as np

import concourse.tile as tile
from concourse import bacc, mybir
from concourse.bass_utils import run_bass_kernel_spmd

N_CORES = 8
B, D, H1, H2, L, K = 32768, 784, 256, 64, 16, 10
P = 112          # partition tile height for the D axis: 784 = 7 * 112
KT = D // P      # 7 k-tiles along D

# per-slot packed weight layout (column offsets in a [128, WSLOT] block)
_E0, _E1, _E2, _D0, _D1, _D2 = 0, 1792, 1920, 1936, 2000, 2256
WSLOT = 3824     # = 7*256 + 2*64 + 16 + 64 + 256 + 2*784
BSLOT = 14       # bias columns per slot: 2 + 1 + 1 + 1 + 2 + 7

# (slots_per_core, rows_per_slot)
_CONFIGS = [(4, 1152), (4, 1280), (8, 640), (16, 320), (32, 160)]

_F32 = mybir.dt.float32
_F32R = mybir.dt.float32r
_BF16 = mybir.dt.bfloat16
_RELU = mybir.ActivationFunctionType.Relu

# matmul-operand dtype mode: "bf16" (default, fast) or "f32r" (precise)
MODE = "bf16"


def _mdt_view(ap, mode):
    return ap.bitcast(_F32R) if mode == "f32r" else ap


def _chunks(R, mode="f32r"):
    """Split R rows into moving-operand chunks <=512.

    f32r: each chunk >=256 (the fp32r full-rate threshold), greedy 512s
    with a rebalanced tail.
    bf16: equal thirds/halves -- every matmul's moving dim stays large
    enough (>=290) that its weight load hides under the stream."""
    if mode == "bf16":
        n = max(1, (R + 511) // 512)
        base, extra = divmod(R, n)
        return [base + (1 if i < extra else 0) for i in range(n)]
    out, rem = [], R
    while rem > 0:
        c = min(512, rem)
        if c == 512 and 0 < rem - c < 256:
            c = max(256, min(512, (rem + 1) // 2))
        out.append(c)
        rem -= c
    return out


def _build_program(S, R, mode):
    mdt = _F32R if mode == "f32r" else _BF16
    idt = _F32 if mode == "f32r" else _BF16
    pipelined = mode == "bf16"
    ncols = S * R
    nflat = ncols * KT  # x/y are stored chunk-flattened: [P, sum(KT*nch)]
    nc = bacc.Bacc("TRN2", target_bir_lowering=False, debug=False)
    xt = nc.dram_tensor("xt", [P, nflat], idt, kind="ExternalInput").ap()
    wp = nc.dram_tensor("wp", [128, S * WSLOT], idt, kind="ExternalInput").ap()
    bp = nc.dram_tensor("bp", [128, S * BSLOT], _F32, kind="ExternalInput").ap()
    yt = nc.dram_tensor("yt", [P, nflat], idt, kind="ExternalOutput").ap()

    chunks = _chunks(R, mode)
    NCH = len(chunks)
    XS_BUFS = 3 if pipelined else 2
    W_BUFS = 4 if pipelined else 2
    H1_BUFS = 10 if pipelined else 6
    SM_BUFS = 6 if pipelined else 3

    with tile.TileContext(nc) as tc:
        with (
            tc.tile_pool(name="wpool", bufs=1) as wpool,
            tc.tile_pool(name="iopool", bufs=1) as iopool,
            tc.tile_pool(name="apool", bufs=1) as apool,
            tc.tile_pool(name="pspool", bufs=1, space="PSUM") as pspool,
        ):
            bsb = wpool.tile([128, S * BSLOT], _F32, tag="b", name="bsb",
                             bufs=1)
            wu = wpool.tile([128, 512], _BF16, tag="wu", name="wu", bufs=1)
            wups = [pspool.tile([128, 512], _F32, tag="ps", name="wups",
                                bufs=7) for _ in range(4)]
            # dedicated keep-warm PSUM target, outside the "ps" rotation:
            # dummy matmuls only ever WAW-chain on the in-order PE.
            dum = pspool.tile([128, 128], _F32, tag="dum", name="dum",
                              bufs=1)

            def dummy(n):
                """n cheap 128-col matmuls: keeps the PE HAM-warm where
                the schedule would otherwise leave the engine idle."""
                for _ in range(n):
                    nc.tensor.matmul(dum, wu[:, 0:128], wu[:, 0:128],
                                     start=True, stop=True)

            def bias(lo, col):
                return bsb[0:lo, col:col + 1]

            def ps_tile(parts, nch):
                return pspool.tile([parts, nch], _F32, tag="ps", name="ps",
                                   bufs=7)

            drain_i = [0]

            def drain_relu(out, ps, bias_ap):
                """bias+ReLU PSUM->SBUF eviction, alternating ACT/DVE."""
                drain_i[0] += 1
                if drain_i[0] % 2:
                    nc.scalar.activation(out, ps, _RELU, bias=bias_ap)
                else:
                    nc.vector.tensor_scalar(out, ps, bias_ap, 0.0,
                                            mybir.AluOpType.add,
                                            mybir.AluOpType.max)

            def drain_bias(out, ps, bias_ap):
                """bias-only PSUM->SBUF eviction, alternating ACT/DVE."""
                drain_i[0] += 1
                if drain_i[0] % 2:
                    nc.scalar.add(out, ps, bias_ap)
                else:
                    nc.vector.tensor_scalar_add(out, ps, bias_ap)

            res = {}
            loff = []
            cum = 0
            for nch in chunks:
                loff.append(cum)
                cum += nch * KT

            def dma_w(s, c0, c1):
                r = res[s]
                nc.sync.dma_start(
                    out=r["w"][:, c0:c1],
                    in_=_mdt_view(wp[:, s * WSLOT + c0:s * WSLOT + c1], mode))

            def dma_x(s, e0, e1):
                r = res[s]
                nc.sync.dma_start(
                    out=r["xs"][0:P, e0:e1],
                    in_=_mdt_view(xt[:, s * R * KT + e0:s * R * KT + e1],
                                  mode))

            def alloc_slot(s):
                if s in res or s >= S:
                    return None
                w = wpool.tile([128, WSLOT], mdt, tag="w", name="w",
                               bufs=W_BUFS)
                xs = iopool.tile([128, R * KT], mdt, tag="xs", name="xs",
                                 bufs=XS_BUFS)
                res[s] = {"w": w, "xs": xs, "base": s * R * KT,
                          "bb": s * BSLOT,
                          "h1": [[None] * NCH, [None] * NCH]}
                return res[s]

            def ensure_slot(s):
                """Steady-state slot prefetch: one wide DMA per tensor."""
                if s in res or s >= S:
                    return
                alloc_slot(s)
                dma_w(s, 0, WSLOT)
                dma_x(s, 0, R * KT)

            def xap(r, ci, k):
                nch = chunks[ci]
                c0 = loff[ci] + k * nch
                return r["xs"][0:P, c0:c0 + nch]

            def e0_unit(s, m, ci):
                """One filler unit: chunk ci's full e0 contraction for
                m-half m (7 matmuls, one short-lived PSUM bank)."""
                r = res[s]
                nch = chunks[ci]
                ps = ps_tile(128, nch)
                for k in range(KT):
                    wk = r["w"][0:P, _E0 + k * 256 + 128 * m:
                                _E0 + k * 256 + 128 * m + 128]
                    nc.tensor.matmul(ps, wk, xap(r, ci, k),
                                     start=(k == 0), stop=(k == KT - 1))
                t = apool.tile([128, nch], mdt, tag="h1", name="h1",
                               bufs=H1_BUFS)
                drain_relu(t, ps, bias(128, r["bb"] + m))
                r["h1"][m][ci] = t

            E0_ORDER = [(m, ci) for m in range(2) for ci in range(NCH)]

            if pipelined:
                # Startup: the DMA queue is in-order and ~270 GB/s, so
                # sequence slot-0/1 pieces just-in-time for the PE.
                alloc_slot(0)
                alloc_slot(1)
                dma_w(0, 0, _E1)                       # s0 e0 weights
                dma_x(0, 0, loff[1] if NCH > 1 else R * KT)   # s0 chunk 0
                nc.sync.dma_start(out=bsb, in_=bp)     # biases (drains)
                dma_w(1, 0, _E1)                       # s1 e0 weights
                for ci in range(1, NCH):               # s0 chunks 1..
                    dma_x(0, loff[ci], loff[ci] + KT * chunks[ci])
                dma_w(0, _E1, _D2)                     # s0 mid weights
                if NCH > 1:                            # s1 chunk 0
                    dma_x(1, 0, loff[1])
                dma_w(0, _D2, WSLOT)                   # s0 d2 weights
                if NCH > 1:                            # s1 chunks 1..
                    dma_x(1, loff[1], R * KT)
                else:
                    dma_x(1, 0, R * KT)
                dma_w(1, _E1, WSLOT)                   # s1 rest

                # PE pre-warm: open the HAM clock gate while DMAs land.
                nc.vector.memset(wu, 0)
                for i in range(12):
                    nc.tensor.matmul(wups[i % 4], wu[:, 0:128], wu,
                                     start=True, stop=True)
                dummy(6)

                # slot 0 runs chunk-major, self-pacing with the head DMAs
                r0 = res[0]
                for ci in range(NCH):
                    for m in range(2):
                        e0_unit(0, m, ci)
                    if ci + 1 < NCH:
                        dummy(6)
            for s in range(S):
                if pipelined:
                    ensure_slot(s + 2)
                    filler = iter(E0_ORDER) if s + 1 < S else iter([])
                else:
                    if s == 0:
                        nc.sync.dma_start(out=bsb, in_=bp)
                    ensure_slot(s)
                    for m, ci in E0_ORDER:
                        e0_unit(s, m, ci)
                    filler = iter([])

                def fill(n, pad=True):
                    for _ in range(n):
                        mk = next(filler, None)
                        if mk is not None:
                            e0_unit(s + 1, *mk)
                        elif pad and pipelined:
                            dummy(3)

                r = res[s]
                w, bb, base, h1 = r["w"], r["bb"], r["base"], r["h1"]

                # encoder 1: [256 -> 64]
                ps = [None] * NCH
                for k in range(2):
                    wk = w[0:128, _E1 + 64 * k:_E1 + 64 * k + 64]
                    for ci, nch in enumerate(chunks):
                        if k == 0:
                            ps[ci] = ps_tile(64, nch)
                        nc.tensor.matmul(ps[ci], wk, h1[k][ci],
                                         start=(k == 0), stop=(k == 1))
                h2 = []
                for ci, nch in enumerate(chunks):
                    t = apool.tile([64, nch], mdt, tag="h2", name="h2", bufs=SM_BUFS)
                    drain_relu(t, ps[ci], bias(64, bb + 2))
                    h2.append(t)
                fill(1)

                # encoder 2: [64 -> 16]
                ps = [None] * NCH
                wk = w[0:64, _E2:_E2 + 16]
                for ci, nch in enumerate(chunks):
                    ps[ci] = ps_tile(16, nch)
                    nc.tensor.matmul(ps[ci], wk, h2[ci], start=True, stop=True)
                z = []
                for ci, nch in enumerate(chunks):
                    t = apool.tile([16, nch], mdt, tag="z", name="z", bufs=SM_BUFS)
                    drain_relu(t, ps[ci], bias(16, bb + 3))
                    z.append(t)
                fill(1)

                # decoder 0: [16 -> 64]
                ps = [None] * NCH
                wk = w[0:16, _D0:_D0 + 64]
                for ci, nch in enumerate(chunks):
                    ps[ci] = ps_tile(64, nch)
                    nc.tensor.matmul(ps[ci], wk, z[ci], start=True, stop=True)
                a1 = []
                for ci, nch in enumerate(chunks):
                    t = apool.tile([64, nch], mdt, tag="a1", name="a1", bufs=SM_BUFS)
                    drain_relu(t, ps[ci], bias(64, bb + 4))
                    a1.append(t)
                fill(1)

                # decoder 1: [64 -> 256]
                a2 = [[None] * NCH, [None] * NCH]
                for m in range(2):
                    wk = w[0:64, _D1 + 128 * m:_D1 + 128 * m + 128]
                    ps = [None] * NCH
                    for ci, nch in enumerate(chunks):
                        ps[ci] = ps_tile(128, nch)
                        nc.tensor.matmul(ps[ci], wk, a1[ci],
                                         start=True, stop=True)
                    for ci, nch in enumerate(chunks):
                        t = apool.tile([128, nch], mdt, tag="a2", name="a2",
                                       bufs=7)
                        drain_relu(t, ps[ci], bias(128, bb + 5 + m))
                        a2[m][ci] = t
                    fill(1)

                # decoder 2: [256 -> 784], bias only.  Chunk-major with a
                # per-chunk y writeback DMA so y flows out continuously
                # instead of piling up after each slot's last drain; the
                # very last chunk's y goes in two half DMAs so only
                # ~0.3 MB trails the kernel's last matmul.
                ys = iopool.tile([128, R * KT], idt, tag="ys", name="ys",
                                 bufs=2)
                for ci, nch in enumerate(chunks):
                    lo = loff[ci]
                    split = (s == S - 1 and ci == NCH - 1)
                    for mm in range(KT):
                        ps = ps_tile(112, nch)
                        for k in range(2):
                            wk = w[0:128, _D2 + 784 * k + 112 * mm:
                                   _D2 + 784 * k + 112 * mm + 112]
                            nc.tensor.matmul(ps, wk, a2[k][ci],
                                             start=(k == 0), stop=(k == 1))
                        drain_bias(ys[0:P, lo + mm * nch:lo + (mm + 1) * nch],
                                   ps, bias(112, bb + 7 + mm))
                        if mm == 3:
                            if split:
                                nc.sync.dma_start(
                                    out=yt[:, base + lo:base + lo + 4 * nch],
                                    in_=ys[0:P, lo:lo + 4 * nch])
                            elif ci < 2:
                                fill(1)
                    if split:
                        nc.sync.dma_start(
                            out=yt[:, base + lo + 4 * nch:
                                   base + lo + KT * nch],
                            in_=ys[0:P, lo + 4 * nch:lo + KT * nch])
                    else:
                        nc.sync.dma_start(
                            out=yt[:, base + lo:base + lo + KT * nch],
                            in_=ys[0:P, lo:lo + KT * nch])
                fill(6, pad=False)
                del res[s]
    nc.compile()
    return nc


_programs = {}


def _get_program(S, R, mode):
    if (S, R, mode) not in _programs:
        _programs[(S, R, mode)] = _build_program(S, R, mode)
    return _programs[(S, R, mode)]


def _pack_weights(params, slot_clusters):
    S = len(slot_clusters)
    wpk = np.zeros((128, S * WSLOT), np.float32)
    bpk = np.zeros((128, S * BSLOT), np.float32)
    for s, c in enumerate(slot_clusters):
        wb, bb = s * WSLOT, s * BSLOT
        we0, we1, we2 = params["w_e0"][c], params["w_e1"][c], params["w_e2"][c]
        wd0, wd1, wd2 = params["w_d0"][c], params["w_d1"][c], params["w_d2"][c]
        for k in range(KT):
            wpk[0:P, wb + _E0 + k * 256: wb + _E0 + (k + 1) * 256] = \
                we0[P * k:P * (k + 1), :]
        for k in range(2):
            wpk[0:128, wb + _E1 + 64 * k: wb + _E1 + 64 * (k + 1)] = \
                we1[128 * k:128 * (k + 1), :]
        wpk[0:64, wb + _E2: wb + _E2 + 16] = we2
        wpk[0:16, wb + _D0: wb + _D0 + 64] = wd0
        wpk[0:64, wb + _D1: wb + _D1 + 256] = wd1
        for k in range(2):
            wpk[0:128, wb + _D2 + 784 * k: wb + _D2 + 784 * (k + 1)] = \
                wd2[128 * k:128 * (k + 1), :]

        be0, be1, be2 = params["b_e0"][c], params["b_e1"][c], params["b_e2"][c]
        bd0, bd1, bd2 = params["b_d0"][c], params["b_d1"][c], params["b_d2"][c]
        bpk[0:128, bb + 0] = be0[0:128]
        bpk[0:128, bb + 1] = be0[128:256]
        bpk[0:64, bb + 2] = be1
        bpk[0:16, bb + 3] = be2
        bpk[0:64, bb + 4] = bd0
        bpk[0:128, bb + 5] = bd1[0:128]
        bpk[0:128, bb + 6] = bd1[128:256]
        for m in range(KT):
            bpk[0:P, bb + 7 + m] = bd2[P * m:P * (m + 1)]
    return wpk, bpk


def _route(labels, mode):
    """Assign rows to (core, slot) blocks; returns config + per-slot rows."""
    counts = np.bincount(labels, minlength=K)
    configs = _CONFIGS if mode == "bf16" else _CONFIGS[1:]
    for S, R in configs:
        need = int(np.sum((counts + R - 1) // R))
        if need <= N_CORES * S:
            break
    nslots = N_CORES * S
    order = np.argsort(labels, kind="stable")
    slot_cluster = np.zeros(nslots, np.int64)
    slot_rows = [np.empty(0, np.int64)] * nslots
    si = pos = 0
    for c in range(K):
        cnt = int(counts[c])
        rows_c = order[pos:pos + cnt]
        pos += cnt
        for off in range(0, cnt, R):
            slot_cluster[si] = c
            slot_rows[si] = rows_c[off:off + R]
            si += 1
    return S, R, slot_cluster, slot_rows


def _flatten_xcore(xcore_t, R, chunks):
    """[D, S*R] feature-major slab -> chunk-flattened [P, S*R*KT]."""
    ncols = xcore_t.shape[1]
    S = ncols // R
    flat = np.empty((P, ncols * KT), np.float32)
    pos = 0
    for s in range(S):
        col = s * R
        for nch in chunks:
            blk = xcore_t[:, col:col + nch]              # [784, nch]
            blk = blk.reshape(KT, P, nch).transpose(1, 0, 2)  # [P, KT, nch]
            flat[:, pos:pos + KT * nch] = blk.reshape(P, KT * nch)
            pos += KT * nch
            col += nch
    return flat


def _unflatten_ycore(yflat, R, chunks):
    """chunk-flattened [P, S*R*KT] -> row-major [S*R, D]."""
    ncols = yflat.shape[1] // KT
    S = ncols // R
    out = np.empty((ncols, D), np.float32)
    pos = 0
    for s in range(S):
        col = s * R
        for nch in chunks:
            blk = yflat[:, pos:pos + KT * nch].reshape(P, KT, nch)
            out[col:col + nch] = blk.transpose(2, 1, 0).reshape(nch, D)
            pos += KT * nch
            col += nch
    return out


def kernel_traced(inputs, trace=False, mode=None):
    if mode is None:
        mode = MODE
    x = np.ascontiguousarray(np.asarray(inputs["x"], dtype=np.float32))
    labels = np.asarray(inputs["kmeans_label"]).astype(np.int64).ravel()
    params = {k: np.asarray(v, dtype=np.float32)
              for k, v in inputs.items() if k not in ("x", "kmeans_label")}

    S, R, slot_cluster, slot_rows = _route(labels, mode)
    chunks = _chunks(R, mode)
    nc = _get_program(S, R, mode)

    in_maps = []
    for i in range(N_CORES):
        xcore = np.zeros((S * R, D), np.float32)
        for s in range(S):
            rows = slot_rows[i * S + s]
            if len(rows):
                xcore[s * R: s * R + len(rows)] = x[rows]
        wpk, bpk = _pack_weights(params, slot_cluster[i * S:(i + 1) * S])
        xflat = _flatten_xcore(np.ascontiguousarray(xcore.T), R, chunks)
        if mode == "bf16":
            import ml_dtypes
            xflat = xflat.astype(ml_dtypes.bfloat16)
            wpk = wpk.astype(ml_dtypes.bfloat16)
        in_maps.append({"xt": xflat, "wp": wpk, "bp": bpk})

    res = run_bass_kernel_spmd(nc, in_maps, core_ids=list(range(N_CORES)),
                               trace=trace)

    out = np.zeros_like(x)
    for i in range(N_CORES):
        yraw = np.asarray(res.results[i]["yt"]).astype(np.float32)
        ytT = _unflatten_ycore(yraw, R, chunks)
        for s in range(S):
            rows = slot_rows[i * S + s]
            if len(rows):
                out[rows] = ytT[s * R: s * R + len(rows)]
    return out, res


def kernel(**inputs):
    out, _ = kernel_traced(inputs, trace=False)
    return out
